# revision 28
# baseline (speedup 1.0000x reference)
"""CausalConv3d FP8 kernel for 8 Trainium2 NeuronCores.

Problem: x (2,128,4,192,320) f32, weight (128,128,3,3,3) f32.
Both are quantized to fp8 e4m3 (round-trip in the reference); conv with
causal temporal padding (2,0) and spatial same padding (1,1),(1,1);
accumulation in fp32. Output (2,128,4,192,320) f32.

Strategy:
  - Host: quantize to fp8 once (bit-identical to the reference's e4m3fn
    round-trip for values < 240, which randn data always is), build a
    zero-padded per-core input slab, and shard (batch=2) x (H into 4
    chunks of 48 rows) across the 8 cores. Weight is replicated.
  - Device (identical SPMD program, per-core data): implicit GEMM.
    For each output row (t, h) the 27 conv taps are 27 accumulated
    128x128 @ 128x320 matmuls into one PSUM bank (Cin = contraction =
    partition dim, Cout = PSUM partition dim, W = free dim). The input
    slab layout [Cin, t_in, h_in, w_pad] makes every tap's rhs a
    contiguous 320-element window.
"""

import numpy as np
import ml_dtypes

import concourse.bass as bass
import concourse.mybir as mybir
from concourse import bacc
from concourse.tile import TileContext, add_dep_helper
from concourse.bass_utils import run_bass_kernel_spmd

F8FN = ml_dtypes.float8_e4m3fn
F8 = ml_dtypes.float8_e4m3  # TRN float8e4; same bits/values as e4m3fn below 240

# Problem shape (hardcoded per contract)
B, C, T, H, W = 2, 128, 4, 192, 320
KT, KH, KW = 3, 3, 3
NTAPS = KT * KH * KW

# Sharding: 8 cores = 2 batches x 4 H-chunks
N_CORES = 8
H_CHUNKS = 4
H_LOC = H // H_CHUNKS  # 48

# DoubleRow tap pairing. The PE executes fp8 pairs (2 taps per matmul,
# K=256 virtual) ~2x faster. Constraints found on HW: the rhs pair
# stride must be positive and EVEN (odd strides are rejected at
# compile), and mixing normal fp8 matmuls with DoubleRow in one program
# can fault the exec unit -- so every matmul is DoubleRow. 27 taps ->
# 13 real pairs + 1 pair whose second weight half is zero (28 entries).
#   - 9 pairs (kt,kh,kw=0)+(kt,kh,kw=2): stride 2
#   - 4 pairs among the kw=1 taps in lex order: strides 336/16128
#   - 1 pair (2,2,1)+ZERO: rhs second half reads off+2 (zero weight)
_KW1 = [(kt, kh, 1) for kt in range(KT) for kh in range(KH)]
TAP_PAIRS = (
    [((kt, kh, 0), (kt, kh, 2)) for kt in range(KT) for kh in range(KH)]
    + [(_KW1[0], _KW1[1]), (_KW1[2], _KW1[3]), (_KW1[4], _KW1[5]), (_KW1[6], _KW1[7])]
    + [(_KW1[8], None)]
)
N_PAIRS = len(TAP_PAIRS)  # 14
NW = 2 * N_PAIRS  # 28 weight entries


def pairs_for_frame(t):
    """Tap pairs for output frame t with causal skipping: taps whose input
    frame r = t + kt - 2 is negative read only zero history and are dropped.
    t=0 -> 5 pairs, t=1 -> 9, t>=2 -> 14 (one zero-padded pair when odd)."""
    kts = [kt for kt in range(KT) if t + kt - 2 >= 0]
    prs = [((kt, kh, 0), (kt, kh, 2)) for kt in kts for kh in range(KH)]
    kw1 = [(kt, kh, 1) for kt in kts for kh in range(KH)]
    for i in range(0, len(kw1) - 1, 2):
        prs.append((kw1[i], kw1[i + 1]))
    if len(kw1) % 2:
        prs.append((kw1[-1], None))
    return prs


# weight table blocks: frame type 0, 1, 2 (frames 2 and 3 share)
_FT_PAIRS = [pairs_for_frame(0), pairs_for_frame(1), pairs_for_frame(2)]
_FT_BASE = [0, len(_FT_PAIRS[0]), len(_FT_PAIRS[0]) + len(_FT_PAIRS[1])]
N_PAIRS_512 = sum(len(p) for p in _FT_PAIRS)  # 28
NW_512 = 2 * N_PAIRS_512  # 56 weight entries

WIN = 512  # psum window (one fp32 bank)

# Per-core padded input slab [C, T_IN, H_IN, W_PAD] fp8
T_IN = T + KT - 1   # 6 (2 leading causal-zero frames + 4 real)
H_IN = H_LOC + KH - 1  # 50 (1 halo row each side)
W_PAD = 322         # 1 left zero + 320 data + 1 trailing zero (even, for
                    # DoubleRow pair strides; the 2-col gap is the kw halo)

ROWS = T * H_LOC          # 192 output rows per core
ROWS_PER_STORE = 8        # rows batched per output DMA

_cache = {}


def _build_nc(h_loc=H_LOC, t=T, rows_per_store=ROWS_PER_STORE, mode="doublerow"):
    """Build the SPMD Bass/Tile program (identical on all cores).

    mode="normal":     27 fp8 matmuls per output row (1 elem/cell/cycle).
    mode="doublerow":  14 all-DoubleRow pair-matmuls per row (TAP_PAIRS).
    """
    t_in = t + KT - 1
    h_in = h_loc + KH - 1
    rows = t * h_loc
    assert rows % rows_per_store == 0

    nw = NW if mode == "doublerow" else NTAPS

    nc = bacc.Bacc(
        "TRN2", target_bir_lowering=False, debug=False, num_devices=N_CORES
    )
    x_d = nc.dram_tensor(
        "x", [C, t_in * h_in * W_PAD], mybir.dt.float8e4, kind="ExternalInput"
    )
    w_d = nc.dram_tensor(
        "w", [C, nw * C], mybir.dt.float8e4, kind="ExternalInput"
    )
    out_d = nc.dram_tensor(
        "out", [C, rows * W], mybir.dt.float32, kind="ExternalOutput"
    )

    def tap_off(tt, hh, tap):
        kt, kh, kw = tap
        return ((tt + kt) * h_in + (hh + kh)) * W_PAD + kw

    with TileContext(nc) as tc:
        with (
            tc.tile_pool(name="xp", bufs=1) as xp,
            tc.tile_pool(name="wp", bufs=1) as wp,
            tc.tile_pool(name="op", bufs=3) as op,
            tc.tile_pool(name="ps", bufs=8, space="PSUM") as pp,
        ):
            w_sb = wp.tile([C, nw, C], mybir.dt.float8e4)
            nc.sync.dma_start(out=w_sb[:], in_=w_d[:].rearrange("p (k c) -> p k c", c=C))

            x_sb = xp.tile([C, t_in * h_in * W_PAD], mybir.dt.float8e4)
            xlen = t_in * h_in * W_PAD
            frame = h_in * W_PAD
            for tf in range(t_in):
                nc.sync.dma_start(
                    out=x_sb[:, tf * frame : (tf + 1) * frame],
                    in_=x_d[:, tf * frame : (tf + 1) * frame],
                )

            def emit_row_normal(tt, hh, ps):
                # weight order: k = (kt*KH + kh)*KW + kw
                taps = [
                    (kt, kh, kw)
                    for kt in range(KT)
                    for kh in range(KH)
                    for kw in range(KW)
                ]
                for k, tap in enumerate(taps):
                    off = tap_off(tt, hh, tap)
                    nc.tensor.matmul(
                        ps[:],
                        w_sb[:, k, :],
                        x_sb[:, off : off + W],
                        start=(k == 0),
                        stop=(k == NTAPS - 1),
                    )

            def emit_row_doublerow(tt, hh, ps):
                for i, (a, b) in enumerate(TAP_PAIRS):
                    off_a = tap_off(tt, hh, a)
                    delta = tap_off(tt, hh, b) - off_a if b is not None else 2
                    assert delta > 0 and delta % 2 == 0, (a, b, delta)
                    rhs = bass.AP(
                        x_sb.tensor,
                        off_a,
                        [[xlen, C], [delta, 2], [1, W]],
                    )
                    nc.tensor.matmul(
                        ps[:],
                        w_sb[:, 2 * i : 2 * i + 2, :],
                        rhs,
                        start=(i == 0),
                        stop=(i == N_PAIRS - 1),
                        perf_mode=mybir.MatmulPerfMode.DoubleRow,
                    )

            emit_row = emit_row_doublerow if mode == "doublerow" else emit_row_normal

            row_list = [(tt, hh) for tt in range(t) for hh in range(h_loc)]
            for gi in range(0, rows, rows_per_store):
                ot = op.tile([C, rows_per_store * W], mybir.dt.float32)
                for j in range(rows_per_store):
                    tt, hh = row_list[gi + j]
                    ps = pp.tile([C, W], mybir.dt.float32)
                    emit_row(tt, hh, ps)
                    nc.vector.tensor_copy(out=ot[:, j * W : (j + 1) * W], in_=ps[:])
                nc.sync.dma_start(
                    out=out_d[:, gi * W : (gi + rows_per_store) * W], in_=ot[:]
                )
    nc.finalize()
    return nc


def _build_nc_512(h_loc=H_LOC, t=T):
    """N=512-window variant: matmuls run over the PADDED output stream
    (48 rows x 336 per frame); the 16-column inter-row pad acts as the
    conv halo, so windows cross row boundaries with no contamination
    (pad outputs are computed and discarded). Causal tap skipping: frame
    t uses only taps with t+kt-2 >= 0 (5/9/14/14 pairs for t=0..3).
    Output goes to DRAM in padded-stream form; host strips the padding.
    """
    frame_in = (h_loc + KH - 1) * W_PAD
    stream = h_loc * W_PAD
    tail = 768
    x0_len = frame_in + tail
    x_len = t * frame_in + tail

    nc = bacc.Bacc(
        "TRN2", target_bir_lowering=False, debug=False, num_devices=N_CORES
    )
    x_d = nc.dram_tensor(
        "x", [C, t * frame_in], mybir.dt.float8e4, kind="ExternalInput"
    )
    w_d = nc.dram_tensor(
        "w", [C, NW_512 * C], mybir.dt.float8e4, kind="ExternalInput"
    )
    out_d = nc.dram_tensor(
        "out", [C, t * stream], mybir.dt.float32, kind="ExternalOutput"
    )

    # exact-fit windows: full 512s plus a tail window covering the stream end
    wins = [(i * WIN, WIN) for i in range(stream // WIN)]
    if stream % WIN:
        wins.append((stream - stream % WIN, stream % WIN))
    out_frame = stream  # out_d holds the exact padded stream per frame

    with TileContext(nc) as tc:
        with (
            tc.tile_pool(name="xp", bufs=1) as xp,
            tc.tile_pool(name="wp", bufs=1) as wp,
            tc.tile_pool(name="op", bufs=7) as op,
            tc.tile_pool(name="ps", bufs=8, space="PSUM") as pp,
        ):
            # frame 0 in its own tile so t=0 windows start as soon as it
            # lands; frames 0-1 likewise for t=1. Bulk x_sb (all frames,
            # for t>=2) loads chained behind them.
            x0_sb = xp.tile([C, x0_len], mybir.dt.float8e4, name="x0_sb", tag="x0")
            nc.any.memset(x0_sb[:, frame_in:], 0.0)
            prev = nc.sync.dma_start(out=x0_sb[:, :frame_in], in_=x_d[:, :frame_in])

            w_sb = wp.tile([C, NW_512, C], mybir.dt.float8e4)
            nc.sync.dma_start(out=w_sb[:], in_=w_d[:].rearrange("p (k c) -> p k c", c=C))

            x01_len = min(2, t) * frame_in + tail
            x01_sb = xp.tile([C, x01_len], mybir.dt.float8e4, name="x01_sb", tag="x01")
            nc.any.memset(x01_sb[:, x01_len - tail :], 0.0)
            d = nc.sync.dma_start(
                out=x01_sb[:, : x01_len - tail], in_=x_d[:, : x01_len - tail]
            )
            add_dep_helper(d.ins, prev.ins, reason="x01 after x0")
            prev = d

            x_sb = xp.tile([C, x_len], mybir.dt.float8e4, name="x_sb", tag="x")
            nc.any.memset(x_sb[:, t * frame_in :], 0.0)
            for tf in range(t):
                d = nc.sync.dma_start(
                    out=x_sb[:, tf * frame_in : (tf + 1) * frame_in],
                    in_=x_d[:, tf * frame_in : (tf + 1) * frame_in],
                )
                add_dep_helper(d.ins, prev.ins, reason="bulk x after x0/x01")

            def roff(t_idx, tap):
                kt, kh, kw = tap
                return (t_idx + kt - 2) * frame_in + kh * W_PAD + kw

            prev_frame_last_mm = None
            for t_idx in range(t):
                ft = min(t_idx, 2)
                prs = _FT_PAIRS[ft]
                base = _FT_BASE[ft]
                if t_idx == 0:
                    xsrc, xsrc_len = x0_sb, x0_len
                elif t_idx == 1:
                    xsrc, xsrc_len = x01_sb, x01_len
                else:
                    xsrc, xsrc_len = x_sb, x_len
                for g0 in range(0, len(wins), 4):
                    grp = wins[g0 : g0 + 4]
                    glen = sum(wl for _, wl in grp)
                    ot = op.tile([C, 4 * WIN], mybir.dt.float32, name="ot", tag="ot")
                    so = 0
                    for p0, wl in grp:
                        ps = pp.tile([C, WIN], mybir.dt.float32, name="ps", tag="ps")
                        for i, (a, b) in enumerate(prs):
                            off_a = roff(t_idx, a) + p0
                            delta = roff(t_idx, b) - roff(t_idx, a) if b else 2
                            assert delta > 0 and delta % 2 == 0
                            rhs = bass.AP(
                                xsrc.tensor,
                                off_a,
                                [[xsrc_len, C], [delta, 2], [1, wl]],
                            )
                            mm = nc.tensor.matmul(
                                ps[:, :wl],
                                w_sb[:, 2 * (base + i) : 2 * (base + i) + 2, :],
                                rhs,
                                start=(i == 0),
                                stop=(i == len(prs) - 1),
                                perf_mode=mybir.MatmulPerfMode.DoubleRow,
                            )
                            if prev_frame_last_mm is not None:
                                # keep the PE stream in frame order so a
                                # later frame's matmul (waiting on its x
                                # DMA) can't head-of-line-block earlier
                                # frames' matmuls
                                add_dep_helper(
                                    mm.ins,
                                    prev_frame_last_mm,
                                    sync=False,
                                    reason="PE stream frame order",
                                )
                                prev_frame_last_mm = None
                            last_mm = mm.ins
                        nc.vector.tensor_copy(
                            out=ot[:, so : so + wl], in_=ps[:, :wl]
                        )
                        so += wl
                    # gpsimd SWDGE: keep output stores off the HWDGE queues
                    # that carry the big input loads, else staging backs up
                    # behind them and stalls the PE via the psum banks
                    nc.gpsimd.dma_start(
                        out=out_d[
                            :,
                            t_idx * out_frame + grp[0][0] : t_idx * out_frame
                            + grp[0][0]
                            + glen,
                        ],
                        in_=ot[:, :glen],
                    )
                prev_frame_last_mm = last_mm
    nc.finalize()
    return nc


DEFAULT_MODE = "dr512"


def get_nc(mode=DEFAULT_MODE):
    key = f"nc_{mode}"
    if key not in _cache:
        if mode == "dr512":
            _cache[key] = _build_nc_512()
        else:
            _cache[key] = _build_nc(mode=mode)
    return _cache[key]


def _prep_inputs(x, weight, mode=DEFAULT_MODE):
    """Quantize + shard. Returns in_maps (list of 8 dicts)."""
    xq = np.asarray(x, dtype=np.float32).astype(F8FN).view(F8)
    wq = np.asarray(weight, dtype=np.float32).astype(F8FN).view(F8)

    # Padded global input: [B, C, T_IN, H+2, W_PAD]; zeros provide the causal
    # temporal pad (2 frames), the H halo at global edges, and the W pad.
    xpad = np.zeros((B, C, T_IN, H + 2, W_PAD), dtype=F8)
    xpad[:, :, KT - 1 :, 1 : H + 1, 1 : W + 1] = xq

    # Weight as lhsT per tap: [Cin, tap, Cout]; tap order must match the
    # emit order in _build_nc for the chosen mode.
    if mode == "dr512":
        w_l = np.zeros((C, NW_512, C), dtype=F8)
        for ft in range(3):
            for i, (a, b) in enumerate(_FT_PAIRS[ft]):
                for j, tap in enumerate((a, b)):
                    if tap is None:
                        continue
                    kt, kh, kw = tap
                    w_l[:, 2 * (_FT_BASE[ft] + i) + j, :] = wq[:, :, kt, kh, kw].T
        w_l = w_l.reshape(C, NW_512 * C)
    elif mode == "doublerow":
        w_l = np.zeros((C, NW, C), dtype=F8)
        for i, (a, b) in enumerate(TAP_PAIRS):
            for j, tap in enumerate((a, b)):
                if tap is None:
                    continue
                kt, kh, kw = tap
                w_l[:, 2 * i + j, :] = wq[:, :, kt, kh, kw].T
        w_l = w_l.reshape(C, NW * C)
    else:
        w_t = wq.transpose(1, 2, 3, 4, 0)  # [Cin, kt, kh, kw, Cout]
        w_l = np.ascontiguousarray(w_t).reshape(C, NTAPS * C)

    tf0 = KT - 1 if mode == "dr512" else 0  # dr512 slab holds real frames only
    in_maps = []
    for b in range(B):
        for hc in range(H_CHUNKS):
            sl = xpad[b, :, tf0:, hc * H_LOC : hc * H_LOC + H_IN, :]
            in_maps.append(
                {"x": np.ascontiguousarray(sl).reshape(C, -1), "w": w_l}
            )
    return in_maps


def _assemble(results, mode=DEFAULT_MODE):
    out = np.empty((B, C, T, H, W), dtype=np.float32)
    i = 0
    for b in range(B):
        for hc in range(H_CHUNKS):
            if mode == "dr512":
                r = results[i]["out"].reshape(C, T, H_LOC, W_PAD)[:, :, :, :W]
            else:
                r = results[i]["out"].reshape(C, T, H_LOC, W)
            out[b, :, :, hc * H_LOC : (hc + 1) * H_LOC, :] = r
            i += 1
    return out


def run(x, weight, trace=False, trace_cores=None, mode=DEFAULT_MODE):
    nc = get_nc(mode)
    in_maps = _prep_inputs(x, weight, mode)
    res = run_bass_kernel_spmd(
        nc,
        in_maps,
        core_ids=list(range(N_CORES)),
        trace=trace,
        trace_cores=trace_cores,
    )
    return _assemble(res.results, mode), res


def kernel(x, weight):
    out, _ = run(x, weight, trace=False)
    return out


# revision 31
# speedup vs baseline: 1.0055x; 1.0055x over previous
"""CausalConv3d FP8 kernel for 8 Trainium2 NeuronCores.

Problem: x (2,128,4,192,320) f32, weight (128,128,3,3,3) f32.
Both are quantized to fp8 e4m3 (round-trip in the reference); conv with
causal temporal padding (2,0) and spatial same padding (1,1),(1,1);
accumulation in fp32. Output (2,128,4,192,320) f32.

Strategy (mode="dr512", ~303 us on HW; 2.4x over the naive implicit GEMM):
  - Host: quantize to fp8 once (bit-identical to the reference's e4m3fn
    round-trip for values < 240, which randn data always is), build a
    zero-padded per-core input slab, and shard (batch=2) x (H into 4
    chunks of 48 rows) across the 8 cores. Weight is replicated.
  - Device (identical SPMD program, per-core data): implicit GEMM with
    Cin as the contraction/partition dim and Cout as the PSUM partition
    dim. Every matmul is an fp8 DoubleRow pair (2 conv taps per
    instruction, K=256 virtual) -- the PE streams 1 column/cycle
    regardless of mode, so pairing taps is the only way to reach the
    2-MAC/cell fp8 rate. Matmuls run N=512-wide over the PADDED output
    stream (48 rows x 322 per frame): the 2-column inter-row pad is
    exactly the kw halo, so windows cross row boundaries with no
    contamination and pad outputs are simply discarded by the host.
    Causal tap skipping drops taps reading the zero history (frames
    t=0/1 need only 5/9 pair-matmuls per window vs 14).
  - Known HW constraints baked in: DoubleRow rhs pair stride must be
    positive and even; never mix normal and DoubleRow matmuls in one
    program (can fault the exec unit); output DMAs go via gpsimd SWDGE
    so they don't queue behind the input loads; explicit PE-stream
    ordering edges keep later frames' matmuls from head-of-line
    blocking earlier ones.
"""

import numpy as np
import ml_dtypes

import concourse.bass as bass
import concourse.mybir as mybir
from concourse import bacc
from concourse.tile import TileContext, add_dep_helper
from concourse.bass_utils import run_bass_kernel_spmd

F8FN = ml_dtypes.float8_e4m3fn
F8 = ml_dtypes.float8_e4m3  # TRN float8e4; same bits/values as e4m3fn below 240

# Problem shape (hardcoded per contract)
B, C, T, H, W = 2, 128, 4, 192, 320
KT, KH, KW = 3, 3, 3
NTAPS = KT * KH * KW

# Sharding: 8 cores = 2 batches x 4 H-chunks
N_CORES = 8
H_CHUNKS = 4
H_LOC = H // H_CHUNKS  # 48

# DoubleRow tap pairing. The PE executes fp8 pairs (2 taps per matmul,
# K=256 virtual) ~2x faster. Constraints found on HW: the rhs pair
# stride must be positive and EVEN (odd strides are rejected at
# compile), and mixing normal fp8 matmuls with DoubleRow in one program
# can fault the exec unit -- so every matmul is DoubleRow. 27 taps ->
# 13 real pairs + 1 pair whose second weight half is zero (28 entries).
#   - 9 pairs (kt,kh,kw=0)+(kt,kh,kw=2): stride 2
#   - 4 pairs among the kw=1 taps in lex order: strides 336/16128
#   - 1 pair (2,2,1)+ZERO: rhs second half reads off+2 (zero weight)
_KW1 = [(kt, kh, 1) for kt in range(KT) for kh in range(KH)]
TAP_PAIRS = (
    [((kt, kh, 0), (kt, kh, 2)) for kt in range(KT) for kh in range(KH)]
    + [(_KW1[0], _KW1[1]), (_KW1[2], _KW1[3]), (_KW1[4], _KW1[5]), (_KW1[6], _KW1[7])]
    + [(_KW1[8], None)]
)
N_PAIRS = len(TAP_PAIRS)  # 14
NW = 2 * N_PAIRS  # 28 weight entries


def pairs_for_frame(t):
    """Tap pairs for output frame t with causal skipping: taps whose input
    frame r = t + kt - 2 is negative read only zero history and are dropped.
    t=0 -> 5 pairs, t=1 -> 9, t>=2 -> 14 (one zero-padded pair when odd)."""
    kts = [kt for kt in range(KT) if t + kt - 2 >= 0]
    prs = [((kt, kh, 0), (kt, kh, 2)) for kt in kts for kh in range(KH)]
    kw1 = [(kt, kh, 1) for kt in kts for kh in range(KH)]
    for i in range(0, len(kw1) - 1, 2):
        prs.append((kw1[i], kw1[i + 1]))
    if len(kw1) % 2:
        prs.append((kw1[-1], None))
    return prs


# weight table blocks: frame type 0, 1, 2 (frames 2 and 3 share)
_FT_PAIRS = [pairs_for_frame(0), pairs_for_frame(1), pairs_for_frame(2)]
_FT_BASE = [0, len(_FT_PAIRS[0]), len(_FT_PAIRS[0]) + len(_FT_PAIRS[1])]
N_PAIRS_512 = sum(len(p) for p in _FT_PAIRS)  # 28
NW_512 = 2 * N_PAIRS_512  # 56 weight entries

WIN = 512  # psum window (one fp32 bank)

# Per-core padded input slab [C, T_IN, H_IN, W_PAD] fp8
T_IN = T + KT - 1   # 6 (2 leading causal-zero frames + 4 real)
H_IN = H_LOC + KH - 1  # 50 (1 halo row each side)
W_PAD = 322         # 1 left zero + 320 data + 1 trailing zero (even, for
                    # DoubleRow pair strides; the 2-col gap is the kw halo)

ROWS = T * H_LOC          # 192 output rows per core
ROWS_PER_STORE = 8        # rows batched per output DMA

_cache = {}


def _build_nc(h_loc=H_LOC, t=T, rows_per_store=ROWS_PER_STORE, mode="doublerow"):
    """Build the SPMD Bass/Tile program (identical on all cores).

    mode="normal":     27 fp8 matmuls per output row (1 elem/cell/cycle).
    mode="doublerow":  14 all-DoubleRow pair-matmuls per row (TAP_PAIRS).
    """
    t_in = t + KT - 1
    h_in = h_loc + KH - 1
    rows = t * h_loc
    assert rows % rows_per_store == 0

    nw = NW if mode == "doublerow" else NTAPS

    nc = bacc.Bacc(
        "TRN2", target_bir_lowering=False, debug=False, num_devices=N_CORES
    )
    x_d = nc.dram_tensor(
        "x", [C, t_in * h_in * W_PAD], mybir.dt.float8e4, kind="ExternalInput"
    )
    w_d = nc.dram_tensor(
        "w", [C, nw * C], mybir.dt.float8e4, kind="ExternalInput"
    )
    out_d = nc.dram_tensor(
        "out", [C, rows * W], mybir.dt.float32, kind="ExternalOutput"
    )

    def tap_off(tt, hh, tap):
        kt, kh, kw = tap
        return ((tt + kt) * h_in + (hh + kh)) * W_PAD + kw

    with TileContext(nc) as tc:
        with (
            tc.tile_pool(name="xp", bufs=1) as xp,
            tc.tile_pool(name="wp", bufs=1) as wp,
            tc.tile_pool(name="op", bufs=3) as op,
            tc.tile_pool(name="ps", bufs=8, space="PSUM") as pp,
        ):
            w_sb = wp.tile([C, nw, C], mybir.dt.float8e4)
            nc.sync.dma_start(out=w_sb[:], in_=w_d[:].rearrange("p (k c) -> p k c", c=C))

            x_sb = xp.tile([C, t_in * h_in * W_PAD], mybir.dt.float8e4)
            xlen = t_in * h_in * W_PAD
            frame = h_in * W_PAD
            for tf in range(t_in):
                nc.sync.dma_start(
                    out=x_sb[:, tf * frame : (tf + 1) * frame],
                    in_=x_d[:, tf * frame : (tf + 1) * frame],
                )

            def emit_row_normal(tt, hh, ps):
                # weight order: k = (kt*KH + kh)*KW + kw
                taps = [
                    (kt, kh, kw)
                    for kt in range(KT)
                    for kh in range(KH)
                    for kw in range(KW)
                ]
                for k, tap in enumerate(taps):
                    off = tap_off(tt, hh, tap)
                    nc.tensor.matmul(
                        ps[:],
                        w_sb[:, k, :],
                        x_sb[:, off : off + W],
                        start=(k == 0),
                        stop=(k == NTAPS - 1),
                    )

            def emit_row_doublerow(tt, hh, ps):
                for i, (a, b) in enumerate(TAP_PAIRS):
                    off_a = tap_off(tt, hh, a)
                    delta = tap_off(tt, hh, b) - off_a if b is not None else 2
                    assert delta > 0 and delta % 2 == 0, (a, b, delta)
                    rhs = bass.AP(
                        x_sb.tensor,
                        off_a,
                        [[xlen, C], [delta, 2], [1, W]],
                    )
                    nc.tensor.matmul(
                        ps[:],
                        w_sb[:, 2 * i : 2 * i + 2, :],
                        rhs,
                        start=(i == 0),
                        stop=(i == N_PAIRS - 1),
                        perf_mode=mybir.MatmulPerfMode.DoubleRow,
                    )

            emit_row = emit_row_doublerow if mode == "doublerow" else emit_row_normal

            row_list = [(tt, hh) for tt in range(t) for hh in range(h_loc)]
            for gi in range(0, rows, rows_per_store):
                ot = op.tile([C, rows_per_store * W], mybir.dt.float32)
                for j in range(rows_per_store):
                    tt, hh = row_list[gi + j]
                    ps = pp.tile([C, W], mybir.dt.float32)
                    emit_row(tt, hh, ps)
                    nc.vector.tensor_copy(out=ot[:, j * W : (j + 1) * W], in_=ps[:])
                nc.sync.dma_start(
                    out=out_d[:, gi * W : (gi + rows_per_store) * W], in_=ot[:]
                )
    nc.finalize()
    return nc


def _build_nc_512(h_loc=H_LOC, t=T):
    """N=512-window variant: matmuls run over the PADDED output stream
    (48 rows x 336 per frame); the 16-column inter-row pad acts as the
    conv halo, so windows cross row boundaries with no contamination
    (pad outputs are computed and discarded). Causal tap skipping: frame
    t uses only taps with t+kt-2 >= 0 (5/9/14/14 pairs for t=0..3).
    Output goes to DRAM in padded-stream form; host strips the padding.
    """
    frame_in = (h_loc + KH - 1) * W_PAD
    stream = h_loc * W_PAD
    tail = 768
    x0_len = frame_in + tail
    x_len = t * frame_in + tail

    nc = bacc.Bacc(
        "TRN2", target_bir_lowering=False, debug=False, num_devices=N_CORES
    )
    x_d = nc.dram_tensor(
        "x", [C, t * frame_in], mybir.dt.float8e4, kind="ExternalInput"
    )
    w_d = nc.dram_tensor(
        "w", [C, NW_512 * C], mybir.dt.float8e4, kind="ExternalInput"
    )
    out_d = nc.dram_tensor(
        "out", [C, t * stream], mybir.dt.float32, kind="ExternalOutput"
    )

    # exact-fit windows: full 512s plus a tail window covering the stream end
    wins = [(i * WIN, WIN) for i in range(stream // WIN)]
    if stream % WIN:
        wins.append((stream - stream % WIN, stream % WIN))
    out_frame = stream  # out_d holds the exact padded stream per frame

    with TileContext(nc) as tc:
        with (
            tc.tile_pool(name="xp", bufs=1) as xp,
            tc.tile_pool(name="wp", bufs=1) as wp,
            tc.tile_pool(name="op", bufs=5) as op,
            tc.tile_pool(name="ps", bufs=8, space="PSUM") as pp,
        ):
            # frame 0 in its own tile so t=0 windows start as soon as it
            # lands; frames 0-1 likewise for t=1. Bulk x_sb (all frames,
            # for t>=2) loads chained behind them.
            x0_sb = xp.tile([C, x0_len], mybir.dt.float8e4, name="x0_sb", tag="x0")
            nc.any.memset(x0_sb[:, frame_in:], 0.0)
            prev = nc.sync.dma_start(out=x0_sb[:, :frame_in], in_=x_d[:, :frame_in])

            w_sb = wp.tile([C, NW_512, C], mybir.dt.float8e4)
            nc.sync.dma_start(out=w_sb[:], in_=w_d[:].rearrange("p (k c) -> p k c", c=C))

            x01_len = min(2, t) * frame_in + tail
            x01_sb = xp.tile([C, x01_len], mybir.dt.float8e4, name="x01_sb", tag="x01")
            nc.any.memset(x01_sb[:, x01_len - tail :], 0.0)
            d = nc.sync.dma_start(
                out=x01_sb[:, : x01_len - tail], in_=x_d[:, : x01_len - tail]
            )
            add_dep_helper(d.ins, prev.ins, reason="x01 after x0")
            prev = d

            x_sb = xp.tile([C, x_len], mybir.dt.float8e4, name="x_sb", tag="x")
            nc.any.memset(x_sb[:, t * frame_in :], 0.0)
            for tf in range(t):
                d = nc.sync.dma_start(
                    out=x_sb[:, tf * frame_in : (tf + 1) * frame_in],
                    in_=x_d[:, tf * frame_in : (tf + 1) * frame_in],
                )
                add_dep_helper(d.ins, prev.ins, reason="bulk x after x0/x01")

            def roff(t_idx, tap):
                kt, kh, kw = tap
                return (t_idx + kt - 2) * frame_in + kh * W_PAD + kw

            prev_frame_last_mm = None
            for t_idx in range(t):
                ft = min(t_idx, 2)
                prs = _FT_PAIRS[ft]
                base = _FT_BASE[ft]
                if t_idx == 0:
                    xsrc, xsrc_len = x0_sb, x0_len
                elif t_idx == 1:
                    xsrc, xsrc_len = x01_sb, x01_len
                else:
                    xsrc, xsrc_len = x_sb, x_len
                for g0 in range(0, len(wins), 4):
                    grp = wins[g0 : g0 + 4]
                    glen = sum(wl for _, wl in grp)
                    ot = op.tile([C, 4 * WIN], mybir.dt.float32, name="ot", tag="ot")
                    so = 0
                    for p0, wl in grp:
                        ps = pp.tile([C, WIN], mybir.dt.float32, name="ps", tag="ps")
                        for i, (a, b) in enumerate(prs):
                            off_a = roff(t_idx, a) + p0
                            delta = roff(t_idx, b) - roff(t_idx, a) if b else 2
                            assert delta > 0 and delta % 2 == 0
                            rhs = bass.AP(
                                xsrc.tensor,
                                off_a,
                                [[xsrc_len, C], [delta, 2], [1, wl]],
                            )
                            mm = nc.tensor.matmul(
                                ps[:, :wl],
                                w_sb[:, 2 * (base + i) : 2 * (base + i) + 2, :],
                                rhs,
                                start=(i == 0),
                                stop=(i == len(prs) - 1),
                                perf_mode=mybir.MatmulPerfMode.DoubleRow,
                            )
                            if prev_frame_last_mm is not None:
                                # keep the PE stream in frame order so a
                                # later frame's matmul (waiting on its x
                                # DMA) can't head-of-line-block earlier
                                # frames' matmuls
                                add_dep_helper(
                                    mm.ins,
                                    prev_frame_last_mm,
                                    sync=False,
                                    reason="PE stream frame order",
                                )
                                prev_frame_last_mm = None
                            last_mm = mm.ins
                        nc.vector.tensor_copy(
                            out=ot[:, so : so + wl], in_=ps[:, :wl]
                        )
                        so += wl
                    # gpsimd SWDGE: keep output stores off the HWDGE queues
                    # that carry the big input loads, else staging backs up
                    # behind them and stalls the PE via the psum banks
                    nc.gpsimd.dma_start(
                        out=out_d[
                            :,
                            t_idx * out_frame + grp[0][0] : t_idx * out_frame
                            + grp[0][0]
                            + glen,
                        ],
                        in_=ot[:, :glen],
                    )
                prev_frame_last_mm = last_mm
    nc.finalize()
    return nc


DEFAULT_MODE = "dr512"


def get_nc(mode=DEFAULT_MODE):
    key = f"nc_{mode}"
    if key not in _cache:
        if mode == "dr512":
            _cache[key] = _build_nc_512()
        else:
            _cache[key] = _build_nc(mode=mode)
    return _cache[key]


def _prep_inputs(x, weight, mode=DEFAULT_MODE):
    """Quantize + shard. Returns in_maps (list of 8 dicts)."""
    xq = np.asarray(x, dtype=np.float32).astype(F8FN).view(F8)
    wq = np.asarray(weight, dtype=np.float32).astype(F8FN).view(F8)

    # Padded global input: [B, C, T_IN, H+2, W_PAD]; zeros provide the causal
    # temporal pad (2 frames), the H halo at global edges, and the W pad.
    xpad = np.zeros((B, C, T_IN, H + 2, W_PAD), dtype=F8)
    xpad[:, :, KT - 1 :, 1 : H + 1, 1 : W + 1] = xq

    # Weight as lhsT per tap: [Cin, tap, Cout]; tap order must match the
    # emit order in _build_nc for the chosen mode.
    if mode == "dr512":
        w_l = np.zeros((C, NW_512, C), dtype=F8)
        for ft in range(3):
            for i, (a, b) in enumerate(_FT_PAIRS[ft]):
                for j, tap in enumerate((a, b)):
                    if tap is None:
                        continue
                    kt, kh, kw = tap
                    w_l[:, 2 * (_FT_BASE[ft] + i) + j, :] = wq[:, :, kt, kh, kw].T
        w_l = w_l.reshape(C, NW_512 * C)
    elif mode == "doublerow":
        w_l = np.zeros((C, NW, C), dtype=F8)
        for i, (a, b) in enumerate(TAP_PAIRS):
            for j, tap in enumerate((a, b)):
                if tap is None:
                    continue
                kt, kh, kw = tap
                w_l[:, 2 * i + j, :] = wq[:, :, kt, kh, kw].T
        w_l = w_l.reshape(C, NW * C)
    else:
        w_t = wq.transpose(1, 2, 3, 4, 0)  # [Cin, kt, kh, kw, Cout]
        w_l = np.ascontiguousarray(w_t).reshape(C, NTAPS * C)

    tf0 = KT - 1 if mode == "dr512" else 0  # dr512 slab holds real frames only
    in_maps = []
    for b in range(B):
        for hc in range(H_CHUNKS):
            sl = xpad[b, :, tf0:, hc * H_LOC : hc * H_LOC + H_IN, :]
            in_maps.append(
                {"x": np.ascontiguousarray(sl).reshape(C, -1), "w": w_l}
            )
    return in_maps


def _assemble(results, mode=DEFAULT_MODE):
    out = np.empty((B, C, T, H, W), dtype=np.float32)
    i = 0
    for b in range(B):
        for hc in range(H_CHUNKS):
            if mode == "dr512":
                r = results[i]["out"].reshape(C, T, H_LOC, W_PAD)[:, :, :, :W]
            else:
                r = results[i]["out"].reshape(C, T, H_LOC, W)
            out[b, :, :, hc * H_LOC : (hc + 1) * H_LOC, :] = r
            i += 1
    return out


def run(x, weight, trace=False, trace_cores=None, mode=DEFAULT_MODE):
    nc = get_nc(mode)
    in_maps = _prep_inputs(x, weight, mode)
    res = run_bass_kernel_spmd(
        nc,
        in_maps,
        core_ids=list(range(N_CORES)),
        trace=trace,
        trace_cores=trace_cores,
    )
    return _assemble(res.results, mode), res


def kernel(x, weight):
    # dr512 is the fast path; fall back to the simpler variants on any
    # unexpected compile/runtime failure.
    for mode in ("dr512", "doublerow", "normal"):
        try:
            out, _ = run(x, weight, trace=False, mode=mode)
            return out
        except Exception:
            if mode == "normal":
                raise
    raise RuntimeError("unreachable")
